# revision 21
# baseline (speedup 1.0000x reference)
"""Trainium2 Bass kernel for an AxialAttentionLayer-style module.

Math: for each batch b,
    scores = s1 + s2,  s1[l,j] = qsum[l]*ksum[j],  s2 = q @ k
    A      = softmax(scores, axis=-1)
    out    = A @ values

Sharding: data-parallel over batch B=32 across 8 cores (4 batches/core).

Default dataflow "t2" (transposed scores, no on-device row max):
    The softmax shift M[l] = max(qsum[l]*Kmax, qsum[l]*Kmin) + 10 is a
    host-computed upper bound on the row max (any shift inside the exp
    safe window works; exact max is unnecessary).  Host also splits
    qsum/ksum into bf16 hi/lo halves so the rank-1 s1 term and the -M
    bias ride a 5-row fp16 matmul at full precision.

    Per 512-column block of L (scores kept transposed as (j, l)):
      MM1a (PE, fp16): sc += keys.T @ qT16         (512 cyc, 1 cyc/col)
      MM1b (PE, fp16): sc += kr5.T @ qrows         (s1 hi/lo + (-M))
      exp  (ACT):      pt = exp(sc)  PSUM -> SBUF f32r (2 blocks/instr,
                       scw=2, amortizes the ~350-cycle ACT overhead)
      MM2  (PE, f32r): op = values.T-stationary @ pt
      copy (DVE):      op PSUM -> SBUF bf16 (pairs of blocks, mm2w=2)
    q loads fp16 (chunked DMAs on SP), out stores bf16 (Pool queue).
    Host divides outT/rowsum in f64 and reassembles (vones trick).

    Perf-critical structure (vs the naive loop):
      * qrows (5 x B_LOC*L fp16, 320 KB) is loop-invariant and lives in
        SBUF as a const tile - one startup DMA instead of 16 small
        per-chunk loads/iter that doubled load-queue occupancy.
      * t2_pipe=1: MM2/copy of superblock i is emitted one superblock
        behind MM1/exp, so PE streams MM1(i+1) while ACT runs exp(i).
      * t2_unroll=16: the For_i hardware loop carries an all-engine
        barrier + semaphore reset per iteration (full pipeline drain);
        unrolling 16 bodies per iteration amortizes it (~45 us/barrier
        at unroll=1 -> ~3 us/body).

Older dataflows "lsoft" (softmax in (l, s) with PE transposes) and "t"
(device row max via gpsimd partition reduce) are kept for reference.
"""

import numpy as np

B, L, S = 32, 8192, 128
N_CORES = 8
B_LOC = B // N_CORES  # 4
LBLK = 512            # l-rows per block
NT = LBLK // 128      # 128-tiles per block
NBLK = L // LBLK      # blocks per batch

_RUNNER_CACHE = {}

# tunables (overridable before building)
CFG = dict(
    qt_bufs=8, p_bufs=4, pt_bufs=4, nm_bufs=4, oc_bufs=6, rs_bufs=2,
    sc_bufs=4, ptps_bufs=2, o_bufs=2,
    out_copy_engine="dve",   # "act" | "dve" | "alt"
    pt_copy_engine="act",    # "act" | "dve" | "alt"
    store_engine="sp",       # "pool" | "act" | "dve" | "sp"
    rowsum_mode="dve",       # "dve" | "pool" | "pe"
    mm1_dtype="f32",         # "f32" | "f32r" | "bf16x2"
    out_dtype="f32",         # "f32" | "bf16"
    bias_mode="act",         # "act" (per-tile exp bias) | "pe" (K=4 accum matmul)
    nm_copy_engine="act",    # "dve" | "act"
    scs_copy_engine="dve",   # "dve" | "act"
    dataflow="t2",           # "lsoft" | "t" | "t2" (transposed, host max-bound)
    rsps_bufs=1,
    max_out_dtype="f32r",    # partition_all_reduce out dtype in "t" flow
    # t2-specific
    t2_rowsum="pe",          # "pe" (sel16 accum matmul) | "pool" (partition_all_reduce)
    t2_sc_bufs=2, t2_o_bufs=2, t2_rsps_bufs=1,
    t2_qt_bufs=6, t2_pt_bufs=6, t2_oc_bufs=6, t2_qr_bufs=4,
    t2_out_copy="dve",       # "dve" | "act"
    t2_chunk=4,              # blocks (of 512 cols) per load/store DMA
    t2_sb=1,                 # blocks per PSUM superblock (sc/exp width)
    t2_pipe=1,               # SW pipeline lag (superblocks) between MM1/exp and rowsum/MM2
    t2_mm2w=2,               # MM2/out-copy width in blocks (1 or 2)
    t2_qr_once=False,        # load all qrows in one DMA per iteration (HW-broken, keep off)
    t2_qr_const=True,        # qrows resident in SBUF as [5, B_LOC*L] const (1 DMA at startup)
    t2_b4=False,             # MM1b as 4 concurrent 32-row-group tiles (needs qr_const)
    t2_dbg_nost=False,       # debug: skip store DMAs only (timing only)
    t2_st_chunk=0,           # blocks per store DMA (0 = t2_chunk)
    t2_scw=2,                # sc/exp width in superblocks (1 or 2); 2 needs pipe>=1
    t2_unroll=16,            # bodies per For_i iteration (largest divisor of repeat used)
    t2_sreset=False,         # staggered semaphore reset in For_i
    t2_q2=False,             # alternate load/store DMA queues (SP/Pool)
    t2_vones=True,           # ones-column in v: rowsum rides MM2, no rowsum matmul
    t2_ab_pair=False,        # with scw=2: emit MM1a,MM1a,MM1b,MM1b (shared stationary)
    t2_dbg_nomm1b=False,     # debug: drop s1/bias matmul (timing only)
    t2_dbg_b_sep=False,      # debug: MM1b into separate unread PSUM (timing only)
    t2_dbg_noio=False,       # debug: skip load/store DMAs (timing only)
    t2_dbg_nopc=False,       # debug: skip PE/ACT/DVE compute (timing only)
)


def _build_nc_t2(repeat=1, cfg=None):
    """Transposed dataflow, no on-device max: scores^T = k^T q^T + rank-1
    (qsum x ksum) - M, where M[l] = max(qsum*Kmax, qsum*Kmin) + 10 is a
    host-side upper bound on the row max (softmax only needs the shift to
    land in the safe exp window, not the exact max).

    Per 512-col block of L:
      MM1a (PE, fp16):  sc(j,l) += k16.T @ qt          (512 cyc)
      MM1b (PE, fp16):  sc(j,l) += kr5.T @ qrows       (s1 hi/lo + bias)
      exp  (ACT):       pt = exp(sc), f32r SBUF
      rowsum:           "pe": sel16 accum matmul into (16,512) PSUM
                        "pool": partition_all_reduce add
      MM2  (PE, f32r):  op(d,l) = v.T-stationary @ pt  (512 cyc)
      copy (DVE):       op PSUM -> oc SBUF bf16
    """
    cfg = {**CFG, **(cfg or {})}
    import concourse.bacc as bacc
    import concourse.mybir as mybir
    import concourse.tile as tile
    from concourse.bass import ts
    from concourse import bass_isa

    f32 = mybir.dt.float32
    f32r = mybir.dt.float32r
    fp16 = mybir.dt.float16
    bf16 = mybir.dt.bfloat16
    Exp = mybir.ActivationFunctionType.Exp

    nc = bacc.Bacc("TRN2", target_bir_lowering=False, debug=False)
    qT16_d = nc.dram_tensor("qT16", (B_LOC, S, L), fp16, kind="ExternalInput")
    if cfg["t2_b4"]:
        assert cfg["t2_qr_const"]
        qrows_d = nc.dram_tensor("qrows101", (128, B_LOC * L), fp16,
                                 kind="ExternalInput")
        kr5_d = nc.dram_tensor("kr5x", (128, B_LOC * S), fp16,
                               kind="ExternalInput")
    elif cfg["t2_qr_const"]:
        qrows_d = nc.dram_tensor("qrows5", (5, B_LOC * L), fp16,
                                 kind="ExternalInput")
        kr5_d = nc.dram_tensor("kr5", (B_LOC, 5, S), fp16, kind="ExternalInput")
    else:
        qrows_d = nc.dram_tensor("qrows", (B_LOC, 5, L), fp16, kind="ExternalInput")
        kr5_d = nc.dram_tensor("kr5", (B_LOC, 5, S), fp16, kind="ExternalInput")
    k16_d = nc.dram_tensor("k16", (B_LOC, S, S), fp16, kind="ExternalInput")
    v_d = nc.dram_tensor("vr", (B_LOC, S, S), f32r, kind="ExternalInput")
    sel16_d = rs_d = None
    if not cfg["t2_vones"]:
        sel16_d = nc.dram_tensor("sel16", (S, NBLK * NBLK), f32r,
                                 kind="ExternalInput")
    outT_d = nc.dram_tensor("outT", (B_LOC, S, L), bf16, kind="ExternalOutput")
    SBv = cfg["t2_sb"]
    if not cfg["t2_vones"]:
        rs_d = nc.dram_tensor("rs", (B_LOC, NBLK // SBv, SBv * LBLK), f32,
                              kind="ExternalOutput")

    with tile.TileContext(nc) as tc:
        with (
            tc.tile_pool(name="const", bufs=1) as constp,
            tc.tile_pool(name="qt", bufs=cfg["t2_qt_bufs"]) as qtp,
            tc.tile_pool(name="qr", bufs=cfg["t2_qr_bufs"]) as qrp,
            tc.tile_pool(name="pt", bufs=cfg["t2_pt_bufs"]) as ptp,
            tc.tile_pool(name="oc", bufs=cfg["t2_oc_bufs"]) as ocp,
            tc.tile_pool(name="rss", bufs=cfg["rs_bufs"]) as rsp,
            tc.tile_pool(name="scps", bufs=cfg["t2_sc_bufs"], space="PSUM") as scps,
            tc.tile_pool(name="ops", bufs=cfg["t2_o_bufs"], space="PSUM") as ops,
            tc.tile_pool(name="rsps", bufs=cfg["t2_rsps_bufs"], space="PSUM") as rspsp,
        ):
            k16_sb = constp.tile([128, B_LOC * 128], fp16, tag="k16")
            if cfg["t2_b4"]:
                kr5_sb = constp.tile([128, B_LOC * 128], fp16, tag="kr5x")
                nc.sync.dma_start(kr5_sb[:], kr5_d[:])
            else:
                kr5_sb = constp.tile([5, B_LOC * 128], fp16, tag="kr5")
            v_r = constp.tile([128, B_LOC * 128], f32r, tag="vr")
            qra_c = None
            if cfg["t2_b4"]:
                qra_c = constp.tile([128, B_LOC * L], fp16, tag="qra_c")
                nc.sync.dma_start(qra_c[:], qrows_d[:])
            elif cfg["t2_qr_const"]:
                qra_c = constp.tile([5, B_LOC * L], fp16, tag="qra_c")
                nc.sync.dma_start(qra_c[:], qrows_d[:])
            sel16_r = None
            if not cfg["t2_vones"]:
                sel16_r = constp.tile([128, NBLK * NBLK], f32r, tag="sel16")
                nc.sync.dma_start(sel16_r[:], sel16_d[:])
            for b in range(B_LOC):
                nc.sync.dma_start(k16_sb[:, ts(b, 128)], k16_d[b])
                if not cfg["t2_b4"]:
                    nc.sync.dma_start(kr5_sb[:, ts(b, 128)], kr5_d[b])
                nc.sync.dma_start(v_r[:, ts(b, 128)], v_d[b])

            CB = cfg["t2_chunk"]          # blocks per load/store DMA chunk
            CL = CB * LBLK                # columns per chunk
            SB = cfg["t2_sb"]             # blocks per PSUM superblock
            SL = SB * LBLK

            NSB = NBLK // SB              # superblocks per batch
            TOT = B_LOC * NSB             # total superblocks
            PIPE = cfg["t2_pipe"]         # back-end lag in superblocks
            STB = cfg["t2_st_chunk"] or CB  # blocks per store DMA
            ST_CL = STB * LBLK
            assert cfg["t2_scw"] == 1 or PIPE >= 1

            def body(_iv=None):
                state = {}                # sb index -> per-sb refs
                chunks = {}               # chunk index -> {qtb,qrb,ocb,c0,b}
                rs_hold = [None]
                op_hold = [None]
                st_hold = [None]
                sc_hold = [None]
                ab_pend = []
                qr_all = [None]

                def front(i):
                    b, sb = divmod(i, NSB)
                    s0 = sb * SL
                    ci = i * SB // CB
                    if cfg["t2_qr_const"]:
                        pass
                    elif cfg["t2_qr_once"] and sb == 0:
                        qra = qrp.tile([5, L], fp16, tag="qra")
                        if not cfg["t2_dbg_noio"]:
                            nc.sync.dma_start(qra[:], qrows_d[b])
                        qr_all[0] = qra
                    if (sb * SB) % CB == 0:
                        c0 = sb * SB // CB * CL
                        qtb = qtp.tile([128, CL], fp16, tag="qtb")
                        qrb = None
                        ldq = (nc.gpsimd if cfg["t2_q2"] and ci % 2
                               else nc.sync)
                        if not cfg["t2_dbg_noio"]:
                            ldq.dma_start(qtb[:],
                                          qT16_d[b, :, c0:c0 + CL])
                        if not (cfg["t2_qr_once"] or cfg["t2_qr_const"]):
                            qrb = qrp.tile([5, CL], fp16, tag="qrb")
                            if not cfg["t2_dbg_noio"]:
                                ldq.dma_start(qrb[:],
                                              qrows_d[b, :, c0:c0 + CL])
                        chunks[ci] = {"qtb": qtb, "qrb": qrb, "c0": c0}
                    ch = chunks[ci]
                    lc = s0 - ch["c0"]
                    SCW = cfg["t2_scw"]
                    w = i % SCW
                    if not cfg["t2_dbg_nopc"]:
                        if w == 0:
                            sc_new = scps.tile([128, SCW * SL], f32,
                                               tag="sc")
                            pt_new = ptp.tile([128, SCW * SL], f32r,
                                              tag="pt")
                            sc_hold[0] = (sc_new, pt_new)
                        sc, ptw = sc_hold[0]
                        one_shot = (cfg["t2_dbg_nomm1b"]
                                    or cfg["t2_dbg_b_sep"]
                                    or cfg["t2_b4"])
                        nc.tensor.matmul(sc[:, ts(w, SL)],
                                         k16_sb[:, ts(b, 128)],
                                         ch["qtb"][:, lc:lc + SL],
                                         start=True,
                                         stop=one_shot)
                        if cfg["t2_qr_const"]:
                            qr_src = qra_c[:, b * L + s0:b * L + s0 + SL]
                        elif cfg["t2_qr_once"]:
                            qr_src = qr_all[0][:, s0:s0 + SL]
                        else:
                            qr_src = ch["qrb"][:, lc:lc + SL]

                        def qr_src_pre(_q=qr_src):
                            return _q
                        if cfg["t2_dbg_b_sep"]:
                            junk = rspsp.tile([128, SL], f32, tag="junk")
                            nc.tensor.matmul(junk[:],
                                             kr5_sb[:, ts(b, 128)],
                                             qr_src_pre(),
                                             start=True, stop=True)
                        elif not cfg["t2_dbg_nomm1b"]:
                            if cfg["t2_b4"]:
                                for t4 in range(4):
                                    p0 = 32 * t4
                                    nc.tensor.matmul(
                                        sc[:, w * SL + t4 * 128:
                                           w * SL + (t4 + 1) * 128],
                                        kr5_sb[p0:p0 + 5, ts(b, 128)],
                                        qra_c[p0:p0 + 5,
                                              b * L + s0 + t4 * 128:
                                              b * L + s0 + (t4 + 1) * 128],
                                        start=False, stop=True,
                                        tile_position=(p0, 0),
                                        skip_group_check=True)
                            elif cfg["t2_ab_pair"] and SCW > 1:
                                ab_pend.append((w, qr_src, b))
                                if w == SCW - 1:
                                    for (wp, qs, bp) in ab_pend:
                                        nc.tensor.matmul(
                                            sc[:, ts(wp, SL)],
                                            kr5_sb[:, ts(bp, 128)],
                                            qs, start=False, stop=True)
                                    ab_pend.clear()
                            else:
                                nc.tensor.matmul(sc[:, ts(w, SL)],
                                                 kr5_sb[:, ts(b, 128)],
                                                 qr_src,
                                                 start=False, stop=True)
                        if w == SCW - 1:
                            nc.scalar.activation(ptw[:], sc[:], Exp,
                                                 bias=0.0, scale=1.0)
                        state[i] = {"pt": ptw, "pto": w * SL, "ci": ci,
                                    "lc": lc}
                    else:
                        ptw = ptp.tile([128, SL], f32r, tag="pt")
                        state[i] = {"pt": ptw, "pto": 0, "ci": ci,
                                    "lc": lc}

                def back(i):
                    b, sb = divmod(i, NSB)
                    st = state.pop(i)
                    ptt, ci, lc = st["pt"], st["ci"], st["lc"]
                    pto = st["pto"]
                    ch = chunks[ci]
                    blk0 = sb * SB
                    if blk0 % STB == 0:
                        oc0 = blk0 * LBLK
                        ocb_new = ocp.tile([128, ST_CL], bf16, tag="ocb")
                        st_hold[0] = (ocb_new, oc0)
                    ocb, oc0 = st_hold[0]
                    so = sb * SL - oc0
                    if sb == 0 and not cfg["t2_vones"]:
                        rs_ps_new = rspsp.tile([NSB, SL], f32, tag="rsps")
                        rs_hold[0] = rs_ps_new
                    rs_ps = rs_hold[0]
                    MW = cfg["t2_mm2w"]
                    assert SB == 1 or MW in (1, SB)
                    if not cfg["t2_dbg_nopc"]:
                        if not cfg["t2_vones"]:
                            nc.tensor.matmul(
                                rs_ps[:],
                                sel16_r[:, NBLK * sb:NBLK * sb + NSB],
                                ptt[:, pto:pto + SL],
                                start=(sb == 0), stop=(sb == NSB - 1))
                        if MW == SB:
                            op_t = ops.tile([128, SL], f32, tag="op")
                            nc.tensor.matmul(op_t[:], v_r[:, ts(b, 128)],
                                             ptt[:, pto:pto + SL],
                                             start=True, stop=True)
                            dst = ocb[:, so:so + SL]
                            if cfg["t2_out_copy"] == "act":
                                nc.scalar.copy(dst, op_t[:])
                            else:
                                nc.vector.tensor_copy(dst, op_t[:])
                        else:
                            for k in range(SB):
                                blk = sb * SB + k
                                m = blk % MW
                                if m == 0:
                                    op_new = ops.tile([128, MW * LBLK], f32,
                                                      tag="op")
                                    op_hold[0] = op_new
                                op_t = op_hold[0]
                                nc.tensor.matmul(
                                    op_t[:, ts(m, LBLK)], v_r[:, ts(b, 128)],
                                    ptt[:, pto + k * LBLK:
                                         pto + (k + 1) * LBLK],
                                    start=True, stop=True)
                                if m == MW - 1:
                                    dst = ocb[:, so + (k - m) * LBLK:
                                              so + (k + 1) * LBLK]
                                    if cfg["t2_out_copy"] == "act":
                                        nc.scalar.copy(dst, op_t[:])
                                    else:
                                        nc.vector.tensor_copy(dst, op_t[:])
                    if (sb == NSB - 1 and not cfg["t2_dbg_nopc"]
                            and not cfg["t2_vones"]):
                        rsx = rsp.tile([NSB, SL], f32, tag="rsx")
                        nc.scalar.copy(rsx[:], rs_ps[:])
                        if not cfg["t2_dbg_noio"]:
                            nc.gpsimd.dma_start(rs_d[b], rsx[:])
                    if ((blk0 + SB) % STB == 0 and not cfg["t2_dbg_noio"]
                            and not cfg["t2_dbg_nost"]):
                        stq = (nc.sync if cfg["t2_q2"]
                               and (blk0 // STB) % 2 else nc.gpsimd)
                        stq.dma_start(
                            outT_d[b, :, oc0:oc0 + ST_CL], ocb[:])
                    if (blk0 + SB) % CB == 0:
                        del chunks[ci]

                for i in range(TOT + PIPE):
                    if i < TOT:
                        front(i)
                    if i >= PIPE:
                        back(i - PIPE)

            if repeat == 1:
                body()
            else:
                un = cfg["t2_unroll"]
                while repeat % un:
                    un -= 1
                with tc.For_i(0, repeat // un, 1,
                              staggered_reset=cfg["t2_sreset"]) as _i:
                    for _ in range(un):
                        body(_i)

    nc.compile()
    return nc


def _build_nc(repeat=1, cfg=None):
    cfg = {**CFG, **(cfg or {})}
    if cfg["dataflow"] == "t2":
        return _build_nc_t2(repeat, cfg)
    import concourse.bacc as bacc
    import concourse.mybir as mybir
    import concourse.tile as tile
    from concourse.bass import ts
    from concourse.masks import make_identity

    f32 = mybir.dt.float32
    f32r = mybir.dt.float32r

    nc = bacc.Bacc("TRN2", target_bir_lowering=False, debug=False)
    bf16 = mybir.dt.bfloat16
    if cfg["mm1_dtype"] == "bf16x2":
        qT_d = nc.dram_tensor("qT2", (B_LOC, S, 2, L), bf16, kind="ExternalInput")
        kph_d = nc.dram_tensor("kph", (B_LOC, S, S), bf16, kind="ExternalInput")
        kpl_d = nc.dram_tensor("kpl", (B_LOC, S, S), bf16, kind="ExternalInput")
    else:
        mm1_dt_glob = f32 if cfg["mm1_dtype"] == "f32" else f32r
        qT_d = nc.dram_tensor("qT", (B_LOC, S, L), mm1_dt_glob, kind="ExternalInput")
    kp_d = nc.dram_tensor("kp", (B_LOC, S, S), f32, kind="ExternalInput")
    v_d = nc.dram_tensor("v", (B_LOC, S, S), f32, kind="ExternalInput")
    ind_d = None
    if cfg["bias_mode"] == "pe":
        ind_d = nc.dram_tensor("ind", (NT, LBLK), f32r, kind="ExternalInput")
    sel16_d = None
    if cfg["dataflow"] == "t" or cfg["rowsum_mode"] == "pe_pt":
        sel16_d = nc.dram_tensor("sel16", (S, NBLK * NBLK), f32r,
                                 kind="ExternalInput")
    out_dt = f32 if cfg["out_dtype"] == "f32" else mybir.dt.bfloat16
    outT_d = nc.dram_tensor("outT", (B_LOC, S, L), out_dt, kind="ExternalOutput")
    if cfg["rowsum_mode"] == "dve" and cfg["dataflow"] == "lsoft":
        rs_d = nc.dram_tensor("rs", (B_LOC, S, L // S), f32, kind="ExternalOutput")
    elif cfg["dataflow"] == "t" or cfg["rowsum_mode"] == "pe_pt":
        rs_d = nc.dram_tensor("rs", (B_LOC, NBLK, LBLK), f32, kind="ExternalOutput")
    else:
        rs_d = nc.dram_tensor("rs", (B_LOC, L), f32, kind="ExternalOutput")

    from concourse import bass_isa
    Exp = mybir.ActivationFunctionType.Exp
    AX = mybir.AxisListType.X
    MAX = mybir.AluOpType.max
    ADD = mybir.AluOpType.add

    with tile.TileContext(nc) as tc:
        with (
            tc.tile_pool(name="const", bufs=1) as constp,
            tc.tile_pool(name="qt", bufs=cfg["qt_bufs"]) as qtp,
            tc.tile_pool(name="p", bufs=cfg["p_bufs"]) as pp,
            tc.tile_pool(name="pt", bufs=cfg["pt_bufs"]) as ptp,
            tc.tile_pool(name="nm", bufs=cfg["nm_bufs"]) as nmp,
            tc.tile_pool(name="rss", bufs=cfg["rs_bufs"]) as rsp,
            tc.tile_pool(name="oc", bufs=cfg["oc_bufs"]) as ocp,
            tc.tile_pool(name="scps", bufs=cfg["sc_bufs"], space="PSUM") as scps,
            tc.tile_pool(name="ptps", bufs=cfg["ptps_bufs"], space="PSUM") as ptps,
            tc.tile_pool(name="ops", bufs=cfg["o_bufs"], space="PSUM") as ops,
            tc.tile_pool(name="rsps", bufs=cfg["rsps_bufs"], space="PSUM") as rspsp,
            tc.tile_pool(name="auxps", bufs=1, space="PSUM") as auxps,
        ):
            ident = constp.tile([128, 128], f32, tag="ident")
            make_identity(nc, ident[:])
            kp_sb = constp.tile([128, B_LOC * 128], f32, tag="kp")
            v_sb = constp.tile([128, B_LOC * 128], f32, tag="v")
            v_r = constp.tile([128, B_LOC * 128], f32r, tag="vr")
            ind_r = None
            if cfg["bias_mode"] == "pe":
                ind_r = constp.tile([NT, LBLK], f32r, tag="ind")
                nc.sync.dma_start(ind_r[:], ind_d[:])
            ones_r = None
            if cfg["rowsum_mode"] == "pe" or cfg["dataflow"] == "t":
                ones_f = constp.tile([128, 1], f32, tag="ones_f")
                ones_r = constp.tile([128, 1], f32r, tag="ones")
                nc.gpsimd.memset(ones_f[:], 1.0)
                nc.vector.tensor_copy(ones_r[:], ones_f[:])
            neg_inv_r = None
            sel16_r = None
            if cfg["rowsum_mode"] == "pe_pt" and cfg["dataflow"] != "t":
                sel16_r = constp.tile([128, NBLK * NBLK], f32r, tag="sel16")
                nc.sync.dma_start(sel16_r[:], sel16_d[:])
            if cfg["dataflow"] == "t":
                neg_inv_f = constp.tile([128, 128], f32, tag="ninv_f")
                neg_inv_r = constp.tile([128, 128], f32r, tag="ninv")
                nc.gpsimd.memset(neg_inv_f[:], -1.0 / 128.0)
                nc.vector.tensor_copy(neg_inv_r[:], neg_inv_f[:])
                sel16_r = constp.tile([128, NBLK * NBLK], f32r, tag="sel16")
                nc.sync.dma_start(sel16_r[:], sel16_d[:])
            for b in range(B_LOC):
                nc.sync.dma_start(kp_sb[:, ts(b, 128)], kp_d[b])
                nc.sync.dma_start(v_sb[:, ts(b, 128)], v_d[b])
            nc.vector.tensor_copy(v_r[:], v_sb[:])
            kp_r = None
            if cfg["mm1_dtype"] == "f32r":
                kp_r = constp.tile([128, B_LOC * 128], f32r, tag="kpr")
                nc.vector.tensor_copy(kp_r[:], kp_sb[:])
            kph_sb = kpl_sb = None
            if cfg["mm1_dtype"] == "bf16x2":
                bf16_ = mybir.dt.bfloat16
                kph_sb = constp.tile([128, B_LOC * 128], bf16_, tag="kph")
                kpl_sb = constp.tile([128, B_LOC * 128], bf16_, tag="kpl")
                for b in range(B_LOC):
                    nc.sync.dma_start(kph_sb[:, ts(b, 128)], kph_d[b])
                    nc.sync.dma_start(kpl_sb[:, ts(b, 128)], kpl_d[b])

            def t_block(b, blk, rs_stage, rs_ps_holder):
                l0 = blk * LBLK
                sc = scps.tile([128, LBLK], f32, tag="sc")
                if cfg["mm1_dtype"] == "bf16x2":
                    bf16_ = mybir.dt.bfloat16
                    qt2 = qtp.tile([128, 2 * LBLK], bf16_, tag="qt")
                    nc.sync.dma_start(
                        qt2[:].rearrange("p (h l) -> p h l", h=2),
                        qT_d[b, :, :, l0:l0 + LBLK])
                    qh = qt2[:, 0:LBLK]
                    ql = qt2[:, LBLK:2 * LBLK]
                    nc.tensor.matmul(sc[:], kph_sb[:, ts(b, 128)], qh,
                                     start=True, stop=False)
                    nc.tensor.matmul(sc[:], kpl_sb[:, ts(b, 128)], qh,
                                     start=False, stop=False)
                    nc.tensor.matmul(sc[:], kph_sb[:, ts(b, 128)], ql,
                                     start=False, stop=False)
                else:
                    mm1_dt = f32 if cfg["mm1_dtype"] == "f32" else f32r
                    kp_use = kp_sb if cfg["mm1_dtype"] == "f32" else kp_r
                    qt = qtp.tile([128, LBLK], mm1_dt, tag="qt")
                    nc.sync.dma_start(qt[:], qT_d[b, :, l0:l0 + LBLK])
                    nc.tensor.matmul(sc[:], kp_use[:, ts(b, 128)], qt[:],
                                     start=True, stop=False)
                scs = pp.tile([128, LBLK], f32, tag="scs")
                if cfg["scs_copy_engine"] == "dve":
                    nc.vector.tensor_copy(scs[:], sc[:])
                else:
                    nc.scalar.copy(scs[:], sc[:])
                mx_dt = f32r if cfg["max_out_dtype"] == "f32r" else f32
                mxr = ptp.tile([128, LBLK], mx_dt, tag="mxr")
                nc.gpsimd.partition_all_reduce(
                    mxr[:], scs[:], 128, bass_isa.ReduceOp.max)
                nc.tensor.matmul(sc[:], neg_inv_r[:], mxr[:],
                                 start=False, stop=True)
                pt = ptp.tile([128, LBLK], f32r, tag="pt")
                nc.scalar.activation(pt[:], sc[:], Exp, bias=0.0, scale=1.0)
                if blk == 0:
                    rs_ps_new = rspsp.tile([NBLK, LBLK], f32, tag="rsps")
                    rs_ps_holder[0] = rs_ps_new
                rs_ps = rs_ps_holder[0]
                nc.tensor.matmul(rs_ps[:], sel16_r[:, blk * NBLK:(blk + 1) * NBLK],
                                 pt[:], start=(blk == 0), stop=(blk == NBLK - 1))
                if blk == NBLK - 1:
                    rsx = rsp.tile([NBLK, LBLK], f32, tag="rsx")
                    nc.vector.tensor_copy(rsx[:], rs_ps[:])
                    nc.sync.dma_start(rs_d[b], rsx[:])
                op_t = ops.tile([128, LBLK], f32, tag="op")
                nc.tensor.matmul(op_t[:], v_r[:, ts(b, 128)], pt[:],
                                 start=True, stop=True)
                oc = ocp.tile([128, LBLK], out_dt, tag="oc")
                oce = cfg["out_copy_engine"]
                if oce in ("alt", "act") or oce.startswith("mix"):
                    nc.scalar.copy(oc[:], op_t[:])
                else:
                    nc.vector.tensor_copy(oc[:], op_t[:])
                st = {"pool": nc.gpsimd, "act": nc.scalar,
                      "dve": nc.vector, "sp": nc.sync}[cfg["store_engine"]]
                st.dma_start(outT_d[b, :, l0:l0 + LBLK], oc[:])

            def t_body(_iv=None):
                for b in range(B_LOC):
                    holder = [None]
                    for blk in range(NBLK):
                        t_block(b, blk, None, holder)

            def body(_iv=None):
                if cfg["dataflow"] == "t":
                    return t_body(_iv)
                for b in range(B_LOC):
                    mode = cfg["rowsum_mode"]
                    rs16_holder = [None]
                    rs_stage = None
                    if mode == "dve":
                        rs_stage = rsp.tile([128, L // S], f32, tag="rss")
                    elif mode == "pool":
                        rs_stage = rsp.tile([128, L], f32, tag="rss")
                    for blk in range(NBLK):
                        l0 = blk * LBLK
                        mm1_dt = f32 if cfg["mm1_dtype"] == "f32" else f32r
                        qt = qtp.tile([128, LBLK], mm1_dt, tag="qt")
                        nc.sync.dma_start(qt[:], qT_d[b, :, l0:l0 + LBLK])
                        sc = scps.tile([128, LBLK], f32, tag="sc")
                        for ti in range(NT):
                            nc.tensor.matmul(
                                sc[:, ts(ti, 128)], qt[:, ts(ti, 128)],
                                (kp_sb if cfg["mm1_dtype"] == "f32" else kp_r)[:, ts(b, 128)],
                                start=True,
                                stop=(cfg["bias_mode"] == "act"),
                                skip_group_check=(cfg["bias_mode"] == "pe"))
                        nm = nmp.tile([128, NT], f32, tag="nm")
                        nc.vector.tensor_reduce(
                            nm[:], sc[:].rearrange("p (t s) -> p t s", t=NT),
                            axis=AX, op=MAX, negate=True)
                        p = pp.tile([128, LBLK], f32, tag="p")
                        if cfg["bias_mode"] == "act":
                            for ti in range(NT):
                                nc.scalar.activation(
                                    p[:, ts(ti, 128)], sc[:, ts(ti, 128)], Exp,
                                    bias=nm[:, ti:ti + 1], scale=1.0)
                        else:
                            nmt_ps = auxps.tile([NT, 128], f32, tag="nmt")
                            nc.tensor.transpose(nmt_ps[:], nm[:], ident[:])
                            nmt = nmp.tile([NT, 128], f32r, tag="nmtr")
                            if cfg["nm_copy_engine"] == "dve":
                                nc.vector.tensor_copy(nmt[:], nmt_ps[:])
                            else:
                                nc.scalar.copy(nmt[:], nmt_ps[:])
                            nc.tensor.matmul(sc[:], nmt[:], ind_r[:],
                                             start=False, stop=True,
                                             skip_group_check=True)
                            nc.scalar.activation(p[:], sc[:], Exp,
                                                 bias=0.0, scale=1.0)
                        if cfg["rowsum_mode"] == "dve":
                            nc.vector.tensor_reduce(
                                rs_stage[:, blk * NT:(blk + 1) * NT],
                                p[:].rearrange("p (t s) -> p t s", t=NT),
                                axis=AX, op=ADD)
                        ptps_t = ptps.tile([128, LBLK], f32, tag="ptps")
                        for ti in range(NT):
                            nc.tensor.transpose(
                                ptps_t[:, ts(ti, 128)], p[:, ts(ti, 128)],
                                ident[:])
                        pt = ptp.tile([128, LBLK], f32r, tag="pt")
                        pce = cfg["pt_copy_engine"]
                        if pce == "alt":
                            pce = "dve" if blk % 2 == 0 else "act"
                        elif pce.startswith("mix"):
                            n, m = pce[3:].split("of")
                            pce = "dve" if blk % int(m) < int(n) else "act"
                        if pce == "dve":
                            nc.vector.tensor_copy(pt[:], ptps_t[:])
                        else:
                            nc.scalar.copy(pt[:], ptps_t[:])
                        if cfg["rowsum_mode"] == "pool":
                            nc.gpsimd.partition_all_reduce(
                                rs_stage[:, blk * LBLK:(blk + 1) * LBLK],
                                pt[:], 128, bass_isa.ReduceOp.add)
                        elif cfg["rowsum_mode"] == "pe":
                            if blk % 4 == 0:
                                rs_ps = rspsp.tile([128, LBLK], f32, tag="rsps")
                            j = blk % 4
                            nc.tensor.matmul(
                                rs_ps[32 * j:32 * j + 1, :], ones_r[:], pt[:],
                                start=True, stop=True,
                                tile_position=(0, 32 * j))
                            if j == 3:
                                nc.vector.tensor_copy(
                                    rs_stage[(blk - 3) // 4 * 4:(blk - 3) // 4 * 4 + 4, :].rearrange("a b -> a b"),
                                    rs_ps[:].rearrange("(a c) b -> a c b", c=32)[:, 0:1, :].rearrange("a c b -> (a c) b"))
                        if cfg["rowsum_mode"] == "pe_pt":
                            if blk == 0:
                                rs16_new = rspsp.tile([NBLK, LBLK], f32,
                                                      tag="rsps")
                                rs16_holder[0] = rs16_new
                            rs16 = rs16_holder[0]
                            nc.tensor.matmul(
                                rs16[:],
                                sel16_r[:, blk * NBLK:(blk + 1) * NBLK],
                                pt[:], start=(blk == 0),
                                stop=(blk == NBLK - 1))
                            if blk == NBLK - 1:
                                rsx = rsp.tile([NBLK, LBLK], f32, tag="rsx")
                                nc.vector.tensor_copy(rsx[:], rs16[:])
                                nc.sync.dma_start(rs_d[b], rsx[:])
                        op_t = ops.tile([128, LBLK], f32, tag="op")
                        nc.tensor.matmul(
                            op_t[:], v_r[:, ts(b, 128)], pt[:],
                            start=True, stop=True)
                        oc = ocp.tile([128, LBLK], out_dt, tag="oc")
                        oce = cfg["out_copy_engine"]
                        if oce == "alt":
                            oce = "act" if blk % 2 == 0 else "dve"
                        elif oce.startswith("mix"):
                            n, m = oce[3:].split("of")
                            oce = "dve" if blk % int(m) < int(n) else "act"
                        if oce == "act":
                            nc.scalar.copy(oc[:], op_t[:])
                        else:
                            nc.vector.tensor_copy(oc[:], op_t[:])
                        st = {"pool": nc.gpsimd, "act": nc.scalar,
                              "dve": nc.vector, "sp": nc.sync}[cfg["store_engine"]]
                        st.dma_start(outT_d[b, :, l0:l0 + LBLK], oc[:])
                    if cfg["rowsum_mode"] == "dve":
                        nc.gpsimd.dma_start(rs_d[b], rs_stage[:])
                    elif cfg["rowsum_mode"] == "pool":
                        nc.sync.dma_start(rs_d[b], rs_stage[0:1, :].rearrange("a b -> (a b)"))

            if repeat == 1:
                body()
            else:
                un = cfg["t2_unroll"]
                while repeat % un:
                    un -= 1
                with tc.For_i(0, repeat // un, 1,
                              staggered_reset=cfg["t2_sreset"]) as _i:
                    for _ in range(un):
                        body(_i)

    nc.compile()
    return nc


def _make_runner(repeat=1, cfg=None):
    """Compile (once) and return fn(in_maps) -> list[dict] per core."""
    key = (repeat, tuple(sorted((cfg or {}).items())))
    if key in _RUNNER_CACHE:
        return _RUNNER_CACHE[key]

    import jax
    import concourse.mybir as mybir
    from concourse import bass2jax
    from concourse.bass2jax import _bass_exec_p, partition_id_tensor
    from jax.sharding import Mesh, NamedSharding, PartitionSpec
    from jax.experimental.shard_map import shard_map

    nc = _build_nc(repeat, cfg)
    bass2jax.install_neuronx_cc_hook()

    in_names, out_names, out_avals, zero_shapes = [], [], [], []
    for alloc in nc.m.functions[0].allocations:
        if not isinstance(alloc, mybir.MemoryLocationSet):
            continue
        name = alloc.memorylocations[0].name
        if alloc.kind == "ExternalInput":
            if nc.partition_id_tensor is None or name != nc.partition_id_tensor.name:
                in_names.append(name)
        elif alloc.kind == "ExternalOutput":
            out_names.append(name)
            shape = tuple(alloc.tensor_shape)
            dtype = mybir.dt.np(alloc.dtype)
            out_avals.append(jax.core.ShapedArray(shape, dtype))
            zero_shapes.append((shape, dtype))
    n_params = len(in_names)
    pid_name = nc.partition_id_tensor.name if nc.partition_id_tensor else None
    names_for_bind = in_names + out_names + ([pid_name] if pid_name else [])

    def _body(*args):
        operands = list(args)
        if pid_name:
            operands.append(partition_id_tensor())
        outs = _bass_exec_p.bind(
            *operands,
            out_avals=tuple(out_avals),
            in_names=tuple(names_for_bind),
            out_names=tuple(out_names),
            lowering_input_output_aliases=(),
            sim_require_finite=True,
            sim_require_nnan=True,
            nc=nc,
        )
        return tuple(outs)

    devices = jax.devices()[:N_CORES]
    mesh = Mesh(np.asarray(devices), ("core",))
    nspec = n_params + len(out_names)
    fn = jax.jit(
        shard_map(_body, mesh=mesh,
                  in_specs=(PartitionSpec("core"),) * nspec,
                  out_specs=(PartitionSpec("core"),) * len(out_names),
                  check_rep=False),
        keep_unused=True)
    sharding = NamedSharding(mesh, PartitionSpec("core"))

    def run(in_maps):
        import jax as _jax
        concat_in = [
            np.concatenate([np.asarray(m[name]) for m in in_maps], axis=0)
            for name in in_names
        ]
        zeros = [np.zeros((N_CORES * s[0],) + tuple(s[1:]), d)
                 for (s, d) in zero_shapes]
        dev_in = [_jax.device_put(a, sharding) for a in concat_in + zeros]
        out_arrs = fn(*dev_in)
        _jax.block_until_ready(out_arrs)
        return [
            {name: np.asarray(out_arrs[i]).reshape(
                (N_CORES,) + tuple(out_avals[i].shape))[c]
             for i, name in enumerate(out_names)}
            for c in range(N_CORES)
        ], (fn, dev_in)

    _RUNNER_CACHE[key] = run
    return run


_VCOL = {}


def _prep_inputs_t2(queries, keys, values, cfg=None):
    cfg = {**CFG, **(cfg or {})}
    import ml_dtypes
    bf = ml_dtypes.bfloat16
    qT = queries.transpose(0, 2, 1)                            # (B, E, L)
    qT16 = np.ascontiguousarray(qT.astype(np.float16))
    k16 = np.ascontiguousarray(keys.astype(np.float16))        # lhsT = keys
    v = np.ascontiguousarray(values.astype(np.float32))
    if cfg["t2_vones"]:
        # ones-column trick: y = V^-1 1; col c* = argmax |y_c|;
        # v[:,c*] := 1 on device, host recovers out_c* = alpha.out + beta*rs
        v64 = values.astype(np.float64)
        cols = np.empty(B, np.int64)
        ys = np.empty((B, S), np.float64)
        for b in range(B):
            y = np.linalg.solve(v64[b], np.ones(S))
            c = int(np.argmax(np.abs(y)))
            cols[b] = c
            ys[b] = y
            v[b, :, c] = 1.0
        _VCOL["cols"] = cols
        _VCOL["ys"] = ys
    qsum = queries.sum(axis=2, dtype=np.float64).astype(np.float32)  # (B, L)
    ksum = keys.sum(axis=2, dtype=np.float64).astype(np.float32)     # (B, S)
    qh = qsum.astype(bf).astype(np.float32)
    ql = qsum - qh
    kh = ksum.astype(bf).astype(np.float32)
    kl = ksum - kh
    Kmax = ksum.max(axis=1, keepdims=True)
    Kmin = ksum.min(axis=1, keepdims=True)
    M = np.maximum(qsum * Kmax, qsum * Kmin) + 10.0
    qrows = np.ascontiguousarray(
        np.stack([qh, ql, qh, -M, ql], axis=1).astype(np.float16))   # (B,5,L)
    ones = np.ones_like(kh)
    kr5 = np.ascontiguousarray(
        np.stack([kh, kh, kl, ones, kl], axis=1).astype(np.float16))  # (B,5,S)
    sel16 = np.zeros((S, NBLK * NBLK), np.float32)
    for j in range(NBLK):
        sel16[:, j * NBLK + j] = 1.0
    in_maps = []
    for c in range(N_CORES):
        sl = slice(c * B_LOC, (c + 1) * B_LOC)
        m = {"qT16": qT16[sl], "k16": k16[sl], "vr": v[sl]}
        if cfg["t2_b4"]:
            qr5 = qrows[sl].transpose(1, 0, 2).reshape(5, B_LOC * L)
            qr101 = np.zeros((128, B_LOC * L), qrows.dtype)
            kr5f = kr5[sl].transpose(1, 0, 2).reshape(5, B_LOC * S)
            kr101 = np.zeros((128, B_LOC * S), kr5.dtype)
            for t in range(4):
                qr101[32 * t:32 * t + 5] = qr5
                kr101[32 * t:32 * t + 5] = kr5f
            m["qrows101"] = np.ascontiguousarray(qr101)
            m["kr5x"] = np.ascontiguousarray(kr101)
        elif cfg["t2_qr_const"]:
            m["qrows5"] = np.ascontiguousarray(
                qrows[sl].transpose(1, 0, 2).reshape(5, B_LOC * L))
            m["kr5"] = kr5[sl]
        else:
            m["qrows"] = qrows[sl]
            m["kr5"] = kr5[sl]
        if not cfg["t2_vones"]:
            m["sel16"] = sel16
        in_maps.append(m)
    return in_maps


def _prep_inputs(queries, keys, values, cfg=None):
    cfg = {**CFG, **(cfg or {})}
    if cfg["dataflow"] == "t2":
        return _prep_inputs_t2(queries, keys, values, cfg)
    qT = np.ascontiguousarray(queries.transpose(0, 2, 1))      # (B, E, L)
    kp = keys + keys.sum(axis=2)[:, None, :]                   # k' = k + 1*ksum
    kp = np.ascontiguousarray(kp.astype(np.float32))
    v = np.ascontiguousarray(values.astype(np.float32))
    qT2 = kph = kpl = None
    if cfg["mm1_dtype"] == "bf16x2":
        import ml_dtypes
        bf = ml_dtypes.bfloat16
        qTh = qT.astype(bf)
        qTl = (qT - qTh.astype(np.float32)).astype(bf)
        qT2 = np.ascontiguousarray(np.stack([qTh, qTl], axis=2))
        kph = kp.astype(bf)
        kpl = np.ascontiguousarray((kp - kph.astype(np.float32)).astype(bf))
        kph = np.ascontiguousarray(kph)
    ind = np.zeros((NT, LBLK), np.float32)
    for ti in range(NT):
        ind[ti, ti * 128:(ti + 1) * 128] = 1.0
    sel16 = np.zeros((S, NBLK * NBLK), np.float32)
    for j in range(NBLK):
        sel16[:, j * NBLK + j] = 1.0
    in_maps = []
    for c in range(N_CORES):
        sl = slice(c * B_LOC, (c + 1) * B_LOC)
        m = {"qT": qT[sl], "kp": kp[sl], "v": v[sl], "ind": ind,
             "sel16": sel16}
        if qT2 is not None:
            m.update({"qT2": qT2[sl], "kph": kph[sl], "kpl": kpl[sl]})
        in_maps.append(m)
    return in_maps


def _assemble(results, cfg=None):
    cfg = {**CFG, **(cfg or {})}
    if cfg["dataflow"] == "t2" and cfg["t2_vones"]:
        return _assemble_t2_vones(results)
    out = np.empty((B, L, S), dtype=np.float32)
    for c in range(N_CORES):
        outT = results[c]["outT"]          # (B_LOC, S, L)  = out^T per batch
        rs = results[c]["rs"]              # (B_LOC, 128, L//128) rowsums
        for b in range(B_LOC):
            if cfg["rowsum_mode"] == "dve" and cfg["dataflow"] == "lsoft":
                rsum = rs[b].T.reshape(L)  # rowsum[l]
            else:
                rsum = rs[b].reshape(L)
            if cfg["dataflow"] == "t2":
                num = outT[b].astype(np.float64).T
                out[c * B_LOC + b] = (num / rsum.astype(np.float64)[:, None]
                                      ).astype(np.float32)
            else:
                out[c * B_LOC + b] = outT[b].T / rsum[:, None]
    return out.reshape(B, 1, L, S)


def _assemble_t2_vones(results):
    out = np.empty((B, L, S), dtype=np.float32)
    cols = _VCOL["cols"]
    ys = _VCOL["ys"]
    for c in range(N_CORES):
        outT = results[c]["outT"]              # (B_LOC, S, L) bf16
        for bl in range(B_LOC):
            b = c * B_LOC + bl
            oT = outT[bl].astype(np.float64)   # (S, L) rows = d
            cc = cols[b]
            y = ys[b]
            rs = oT[cc]                        # rowsum row
            beta = 1.0 / y[cc]
            alpha = -y * beta
            alpha[cc] = 0.0
            rec = alpha @ oT + beta * rs       # recovered out_cc (L,)
            o = oT / rs[None, :]
            o[cc] = rec / rs
            out[b] = o.T.astype(np.float32)
    return out.reshape(B, 1, L, S)


def kernel(queries, keys, values):
    run = _make_runner(repeat=1)
    in_maps = _prep_inputs(queries, keys, values)
    results, _ = run(in_maps)
    return _assemble(results)



# revision 22
# speedup vs baseline: 1.2357x; 1.2357x over previous
"""Trainium2 Bass kernel for an AxialAttentionLayer-style module.

Math: for each batch b,
    scores = s1 + s2,  s1[l,j] = qsum[l]*ksum[j],  s2 = q @ k
    A      = softmax(scores, axis=-1)
    out    = A @ values

Sharding: data-parallel over batch B=32 across 8 cores (4 batches/core).

Default dataflow "t2" (transposed scores, no on-device row max):
    The softmax shift M[l] = max(qsum[l]*Kmax, qsum[l]*Kmin) + 10 is a
    host-computed upper bound on the row max (any shift inside the exp
    safe window works; exact max is unnecessary).  Host also splits
    qsum/ksum into bf16 hi/lo halves so the rank-1 s1 term and the -M
    bias ride a 5-row fp16 matmul at full precision.

    Per 512-column block of L (scores kept transposed as (j, l)):
      MM1a (PE, fp16): sc += keys.T @ qT16         (512 cyc, 1 cyc/col)
      MM1b (PE, fp16): sc += kr5.T @ qrows         (s1 hi/lo + (-M))
      exp  (ACT):      pt = exp(sc)  PSUM -> SBUF f32r (2 blocks/instr,
                       scw=2, amortizes the ~350-cycle ACT overhead)
      MM2  (PE, f32r): op = values.T-stationary @ pt
      copy (DVE):      op PSUM -> SBUF bf16 (pairs of blocks, mm2w=2)
    q loads fp16 (chunked DMAs on SP), out stores bf16 (Pool queue).
    Host divides outT/rowsum in f64 and reassembles (vones trick).

    Perf-critical structure (vs the naive loop):
      * qrows (5 x B_LOC*L fp16, 320 KB) is loop-invariant and lives in
        SBUF as a const tile - one startup DMA instead of 16 small
        per-chunk loads/iter that doubled load-queue occupancy.
      * t2_pipe=1: MM2/copy of superblock i is emitted one superblock
        behind MM1/exp, so PE streams MM1(i+1) while ACT runs exp(i).
      * t2_unroll=16: the For_i hardware loop carries an all-engine
        barrier + semaphore reset per iteration (full pipeline drain);
        unrolling 16 bodies per iteration amortizes it (~45 us/barrier
        at unroll=1 -> ~3 us/body).

Older dataflows "lsoft" (softmax in (l, s) with PE transposes) and "t"
(device row max via gpsimd partition reduce) are kept for reference.
"""

import numpy as np

B, L, S = 32, 8192, 128
N_CORES = 8
B_LOC = B // N_CORES  # 4
LBLK = 512            # l-rows per block
NT = LBLK // 128      # 128-tiles per block
NBLK = L // LBLK      # blocks per batch

_RUNNER_CACHE = {}

# tunables (overridable before building)
CFG = dict(
    qt_bufs=8, p_bufs=4, pt_bufs=4, nm_bufs=4, oc_bufs=6, rs_bufs=2,
    sc_bufs=4, ptps_bufs=2, o_bufs=2,
    out_copy_engine="dve",   # "act" | "dve" | "alt"
    pt_copy_engine="act",    # "act" | "dve" | "alt"
    store_engine="sp",       # "pool" | "act" | "dve" | "sp"
    rowsum_mode="dve",       # "dve" | "pool" | "pe"
    mm1_dtype="f32",         # "f32" | "f32r" | "bf16x2"
    out_dtype="f32",         # "f32" | "bf16"
    bias_mode="act",         # "act" (per-tile exp bias) | "pe" (K=4 accum matmul)
    nm_copy_engine="act",    # "dve" | "act"
    scs_copy_engine="dve",   # "dve" | "act"
    dataflow="t2",           # "lsoft" | "t" | "t2" (transposed, host max-bound)
    rsps_bufs=1,
    max_out_dtype="f32r",    # partition_all_reduce out dtype in "t" flow
    # t2-specific
    t2_rowsum="pe",          # "pe" (sel16 accum matmul) | "pool" (partition_all_reduce)
    t2_sc_bufs=2, t2_o_bufs=2, t2_rsps_bufs=1,
    t2_qt_bufs=6, t2_pt_bufs=6, t2_oc_bufs=6, t2_qr_bufs=4,
    t2_out_copy="dve",       # "dve" | "act"
    t2_chunk=4,              # blocks (of 512 cols) per load/store DMA
    t2_sb=1,                 # blocks per PSUM superblock (sc/exp width)
    t2_pipe=1,               # SW pipeline lag (superblocks) between MM1/exp and rowsum/MM2
    t2_mm2w=2,               # MM2/out-copy width in blocks (1 or 2)
    t2_qr_once=False,        # load all qrows in one DMA per iteration (HW-broken, keep off)
    t2_qr_const=True,        # qrows resident in SBUF as [5, B_LOC*L] const (1 DMA at startup)
    t2_b4=False,             # MM1b as concurrent row-group tiles (needs qr_const)
    t2_b4n=4,                # number of MM1b row-group tiles (2 or 4)
    t2_dbg_nost=False,       # debug: skip store DMAs only (timing only)
    t2_st_chunk=0,           # blocks per store DMA (0 = t2_chunk)
    t2_scw=2,                # sc/exp width in superblocks (1 or 2); 2 needs pipe>=1
    t2_unroll=16,            # bodies per For_i iteration (largest divisor of repeat used)
    t2_sreset=False,         # staggered semaphore reset in For_i
    t2_q2=False,             # alternate load/store DMA queues (SP/Pool)
    t2_vones=True,           # ones-column in v: rowsum rides MM2, no rowsum matmul
    t2_ab_pair=False,        # with scw=2: emit MM1a,MM1a,MM1b,MM1b (shared stationary)
    t2_dbg_nomm1b=False,     # debug: drop s1/bias matmul (timing only)
    t2_dbg_b_sep=False,      # debug: MM1b into separate unread PSUM (timing only)
    t2_dbg_noio=False,       # debug: skip load/store DMAs (timing only)
    t2_dbg_nopc=False,       # debug: skip PE/ACT/DVE compute (timing only)
)


def _build_nc_t2(repeat=1, cfg=None):
    """Transposed dataflow, no on-device max: scores^T = k^T q^T + rank-1
    (qsum x ksum) - M, where M[l] = max(qsum*Kmax, qsum*Kmin) + 10 is a
    host-side upper bound on the row max (softmax only needs the shift to
    land in the safe exp window, not the exact max).

    Per 512-col block of L:
      MM1a (PE, fp16):  sc(j,l) += k16.T @ qt          (512 cyc)
      MM1b (PE, fp16):  sc(j,l) += kr5.T @ qrows       (s1 hi/lo + bias)
      exp  (ACT):       pt = exp(sc), f32r SBUF
      rowsum:           "pe": sel16 accum matmul into (16,512) PSUM
                        "pool": partition_all_reduce add
      MM2  (PE, f32r):  op(d,l) = v.T-stationary @ pt  (512 cyc)
      copy (DVE):       op PSUM -> oc SBUF bf16
    """
    cfg = {**CFG, **(cfg or {})}
    import concourse.bacc as bacc
    import concourse.mybir as mybir
    import concourse.tile as tile
    from concourse.bass import ts
    from concourse import bass_isa

    f32 = mybir.dt.float32
    f32r = mybir.dt.float32r
    fp16 = mybir.dt.float16
    bf16 = mybir.dt.bfloat16
    Exp = mybir.ActivationFunctionType.Exp

    nc = bacc.Bacc("TRN2", target_bir_lowering=False, debug=False)
    qT16_d = nc.dram_tensor("qT16", (B_LOC, S, L), fp16, kind="ExternalInput")
    if cfg["t2_b4"]:
        assert cfg["t2_qr_const"]
        qrows_d = nc.dram_tensor("qrows101", (128, B_LOC * L), fp16,
                                 kind="ExternalInput")
        kr5_d = nc.dram_tensor("kr5x", (128, B_LOC * S), fp16,
                               kind="ExternalInput")
    elif cfg["t2_qr_const"]:
        qrows_d = nc.dram_tensor("qrows5", (5, B_LOC * L), fp16,
                                 kind="ExternalInput")
        kr5_d = nc.dram_tensor("kr5", (B_LOC, 5, S), fp16, kind="ExternalInput")
    else:
        qrows_d = nc.dram_tensor("qrows", (B_LOC, 5, L), fp16, kind="ExternalInput")
        kr5_d = nc.dram_tensor("kr5", (B_LOC, 5, S), fp16, kind="ExternalInput")
    k16_d = nc.dram_tensor("k16", (B_LOC, S, S), fp16, kind="ExternalInput")
    v_d = nc.dram_tensor("vr", (B_LOC, S, S), f32r, kind="ExternalInput")
    sel16_d = rs_d = None
    if not cfg["t2_vones"]:
        sel16_d = nc.dram_tensor("sel16", (S, NBLK * NBLK), f32r,
                                 kind="ExternalInput")
    outT_d = nc.dram_tensor("outT", (B_LOC, S, L), bf16, kind="ExternalOutput")
    SBv = cfg["t2_sb"]
    if not cfg["t2_vones"]:
        rs_d = nc.dram_tensor("rs", (B_LOC, NBLK // SBv, SBv * LBLK), f32,
                              kind="ExternalOutput")

    with tile.TileContext(nc) as tc:
        with (
            tc.tile_pool(name="const", bufs=1) as constp,
            tc.tile_pool(name="qt", bufs=cfg["t2_qt_bufs"]) as qtp,
            tc.tile_pool(name="qr", bufs=cfg["t2_qr_bufs"]) as qrp,
            tc.tile_pool(name="pt", bufs=cfg["t2_pt_bufs"]) as ptp,
            tc.tile_pool(name="oc", bufs=cfg["t2_oc_bufs"]) as ocp,
            tc.tile_pool(name="rss", bufs=cfg["rs_bufs"]) as rsp,
            tc.tile_pool(name="scps", bufs=cfg["t2_sc_bufs"], space="PSUM") as scps,
            tc.tile_pool(name="ops", bufs=cfg["t2_o_bufs"], space="PSUM") as ops,
            tc.tile_pool(name="rsps", bufs=cfg["t2_rsps_bufs"], space="PSUM") as rspsp,
        ):
            k16_sb = constp.tile([128, B_LOC * 128], fp16, tag="k16")
            if cfg["t2_b4"]:
                kr5_sb = constp.tile([128, B_LOC * 128], fp16, tag="kr5x")
                nc.sync.dma_start(kr5_sb[:], kr5_d[:])
            else:
                kr5_sb = constp.tile([5, B_LOC * 128], fp16, tag="kr5")
            v_r = constp.tile([128, B_LOC * 128], f32r, tag="vr")
            qra_c = None
            if cfg["t2_b4"]:
                qra_c = constp.tile([128, B_LOC * L], fp16, tag="qra_c")
                nc.sync.dma_start(qra_c[:], qrows_d[:])
            elif cfg["t2_qr_const"]:
                qra_c = constp.tile([5, B_LOC * L], fp16, tag="qra_c")
                nc.sync.dma_start(qra_c[:], qrows_d[:])
            sel16_r = None
            if not cfg["t2_vones"]:
                sel16_r = constp.tile([128, NBLK * NBLK], f32r, tag="sel16")
                nc.sync.dma_start(sel16_r[:], sel16_d[:])
            for b in range(B_LOC):
                nc.sync.dma_start(k16_sb[:, ts(b, 128)], k16_d[b])
                if not cfg["t2_b4"]:
                    nc.sync.dma_start(kr5_sb[:, ts(b, 128)], kr5_d[b])
                nc.sync.dma_start(v_r[:, ts(b, 128)], v_d[b])

            CB = cfg["t2_chunk"]          # blocks per load/store DMA chunk
            CL = CB * LBLK                # columns per chunk
            SB = cfg["t2_sb"]             # blocks per PSUM superblock
            SL = SB * LBLK

            NSB = NBLK // SB              # superblocks per batch
            TOT = B_LOC * NSB             # total superblocks
            PIPE = cfg["t2_pipe"]         # back-end lag in superblocks
            STB = cfg["t2_st_chunk"] or CB  # blocks per store DMA
            ST_CL = STB * LBLK
            assert cfg["t2_scw"] == 1 or PIPE >= 1

            def body(_iv=None):
                state = {}                # sb index -> per-sb refs
                chunks = {}               # chunk index -> {qtb,qrb,ocb,c0,b}
                rs_hold = [None]
                op_hold = [None]
                st_hold = [None]
                sc_hold = [None]
                ab_pend = []
                qr_all = [None]

                def front(i):
                    b, sb = divmod(i, NSB)
                    s0 = sb * SL
                    ci = i * SB // CB
                    if cfg["t2_qr_const"]:
                        pass
                    elif cfg["t2_qr_once"] and sb == 0:
                        qra = qrp.tile([5, L], fp16, tag="qra")
                        if not cfg["t2_dbg_noio"]:
                            nc.sync.dma_start(qra[:], qrows_d[b])
                        qr_all[0] = qra
                    if (sb * SB) % CB == 0:
                        c0 = sb * SB // CB * CL
                        qtb = qtp.tile([128, CL], fp16, tag="qtb")
                        qrb = None
                        ldq = (nc.gpsimd if cfg["t2_q2"] and ci % 2
                               else nc.sync)
                        if not cfg["t2_dbg_noio"]:
                            ldq.dma_start(qtb[:],
                                          qT16_d[b, :, c0:c0 + CL])
                        if not (cfg["t2_qr_once"] or cfg["t2_qr_const"]):
                            qrb = qrp.tile([5, CL], fp16, tag="qrb")
                            if not cfg["t2_dbg_noio"]:
                                ldq.dma_start(qrb[:],
                                              qrows_d[b, :, c0:c0 + CL])
                        chunks[ci] = {"qtb": qtb, "qrb": qrb, "c0": c0}
                    ch = chunks[ci]
                    lc = s0 - ch["c0"]
                    SCW = cfg["t2_scw"]
                    w = i % SCW
                    if not cfg["t2_dbg_nopc"]:
                        if w == 0:
                            sc_new = scps.tile([128, SCW * SL], f32,
                                               tag="sc")
                            pt_new = ptp.tile([128, SCW * SL], f32r,
                                              tag="pt")
                            sc_hold[0] = (sc_new, pt_new)
                        sc, ptw = sc_hold[0]
                        one_shot = (cfg["t2_dbg_nomm1b"]
                                    or cfg["t2_dbg_b_sep"]
                                    or cfg["t2_b4"])
                        nc.tensor.matmul(sc[:, ts(w, SL)],
                                         k16_sb[:, ts(b, 128)],
                                         ch["qtb"][:, lc:lc + SL],
                                         start=True,
                                         stop=one_shot)
                        if cfg["t2_qr_const"]:
                            qr_src = qra_c[:, b * L + s0:b * L + s0 + SL]
                        elif cfg["t2_qr_once"]:
                            qr_src = qr_all[0][:, s0:s0 + SL]
                        else:
                            qr_src = ch["qrb"][:, lc:lc + SL]

                        def qr_src_pre(_q=qr_src):
                            return _q
                        if cfg["t2_dbg_b_sep"]:
                            junk = rspsp.tile([128, SL], f32, tag="junk")
                            nc.tensor.matmul(junk[:],
                                             kr5_sb[:, ts(b, 128)],
                                             qr_src_pre(),
                                             start=True, stop=True)
                        elif not cfg["t2_dbg_nomm1b"]:
                            if cfg["t2_b4"]:
                                n4 = cfg["t2_b4n"]
                                w4 = SL // n4
                                for t4 in range(n4):
                                    p0 = (128 // n4) * t4
                                    nc.tensor.matmul(
                                        sc[:, w * SL + t4 * w4:
                                           w * SL + (t4 + 1) * w4],
                                        kr5_sb[p0:p0 + 5, ts(b, 128)],
                                        qra_c[p0:p0 + 5,
                                              b * L + s0 + t4 * w4:
                                              b * L + s0 + (t4 + 1) * w4],
                                        start=False, stop=True,
                                        tile_position=(p0, 0),
                                        skip_group_check=True)
                            elif cfg["t2_ab_pair"] and SCW > 1:
                                ab_pend.append((w, qr_src, b))
                                if w == SCW - 1:
                                    for (wp, qs, bp) in ab_pend:
                                        nc.tensor.matmul(
                                            sc[:, ts(wp, SL)],
                                            kr5_sb[:, ts(bp, 128)],
                                            qs, start=False, stop=True)
                                    ab_pend.clear()
                            else:
                                nc.tensor.matmul(sc[:, ts(w, SL)],
                                                 kr5_sb[:, ts(b, 128)],
                                                 qr_src,
                                                 start=False, stop=True)
                        if w == SCW - 1:
                            nc.scalar.activation(ptw[:], sc[:], Exp,
                                                 bias=0.0, scale=1.0)
                        state[i] = {"pt": ptw, "pto": w * SL, "ci": ci,
                                    "lc": lc}
                    else:
                        ptw = ptp.tile([128, SL], f32r, tag="pt")
                        state[i] = {"pt": ptw, "pto": 0, "ci": ci,
                                    "lc": lc}

                def back(i):
                    b, sb = divmod(i, NSB)
                    st = state.pop(i)
                    ptt, ci, lc = st["pt"], st["ci"], st["lc"]
                    pto = st["pto"]
                    ch = chunks[ci]
                    blk0 = sb * SB
                    if blk0 % STB == 0:
                        oc0 = blk0 * LBLK
                        ocb_new = ocp.tile([128, ST_CL], bf16, tag="ocb")
                        st_hold[0] = (ocb_new, oc0)
                    ocb, oc0 = st_hold[0]
                    so = sb * SL - oc0
                    if sb == 0 and not cfg["t2_vones"]:
                        rs_ps_new = rspsp.tile([NSB, SL], f32, tag="rsps")
                        rs_hold[0] = rs_ps_new
                    rs_ps = rs_hold[0]
                    MW = cfg["t2_mm2w"]
                    assert SB == 1 or MW in (1, SB)
                    if not cfg["t2_dbg_nopc"]:
                        if not cfg["t2_vones"]:
                            nc.tensor.matmul(
                                rs_ps[:],
                                sel16_r[:, NBLK * sb:NBLK * sb + NSB],
                                ptt[:, pto:pto + SL],
                                start=(sb == 0), stop=(sb == NSB - 1))
                        if MW == SB:
                            op_t = ops.tile([128, SL], f32, tag="op")
                            nc.tensor.matmul(op_t[:], v_r[:, ts(b, 128)],
                                             ptt[:, pto:pto + SL],
                                             start=True, stop=True)
                            dst = ocb[:, so:so + SL]
                            if cfg["t2_out_copy"] == "act":
                                nc.scalar.copy(dst, op_t[:])
                            else:
                                nc.vector.tensor_copy(dst, op_t[:])
                        else:
                            for k in range(SB):
                                blk = sb * SB + k
                                m = blk % MW
                                if m == 0:
                                    op_new = ops.tile([128, MW * LBLK], f32,
                                                      tag="op")
                                    op_hold[0] = op_new
                                op_t = op_hold[0]
                                nc.tensor.matmul(
                                    op_t[:, ts(m, LBLK)], v_r[:, ts(b, 128)],
                                    ptt[:, pto + k * LBLK:
                                         pto + (k + 1) * LBLK],
                                    start=True, stop=True)
                                if m == MW - 1:
                                    dst = ocb[:, so + (k - m) * LBLK:
                                              so + (k + 1) * LBLK]
                                    if cfg["t2_out_copy"] == "act":
                                        nc.scalar.copy(dst, op_t[:])
                                    else:
                                        nc.vector.tensor_copy(dst, op_t[:])
                    if (sb == NSB - 1 and not cfg["t2_dbg_nopc"]
                            and not cfg["t2_vones"]):
                        rsx = rsp.tile([NSB, SL], f32, tag="rsx")
                        nc.scalar.copy(rsx[:], rs_ps[:])
                        if not cfg["t2_dbg_noio"]:
                            nc.gpsimd.dma_start(rs_d[b], rsx[:])
                    if ((blk0 + SB) % STB == 0 and not cfg["t2_dbg_noio"]
                            and not cfg["t2_dbg_nost"]):
                        stq = (nc.sync if cfg["t2_q2"]
                               and (blk0 // STB) % 2 else nc.gpsimd)
                        stq.dma_start(
                            outT_d[b, :, oc0:oc0 + ST_CL], ocb[:])
                    if (blk0 + SB) % CB == 0:
                        del chunks[ci]

                for i in range(TOT + PIPE):
                    if i < TOT:
                        front(i)
                    if i >= PIPE:
                        back(i - PIPE)

            if repeat == 1:
                body()
            else:
                un = cfg["t2_unroll"]
                while repeat % un:
                    un -= 1
                with tc.For_i(0, repeat // un, 1,
                              staggered_reset=cfg["t2_sreset"]) as _i:
                    for _ in range(un):
                        body(_i)

    nc.compile()
    return nc


def _build_nc(repeat=1, cfg=None):
    cfg = {**CFG, **(cfg or {})}
    if cfg["dataflow"] == "t2":
        return _build_nc_t2(repeat, cfg)
    import concourse.bacc as bacc
    import concourse.mybir as mybir
    import concourse.tile as tile
    from concourse.bass import ts
    from concourse.masks import make_identity

    f32 = mybir.dt.float32
    f32r = mybir.dt.float32r

    nc = bacc.Bacc("TRN2", target_bir_lowering=False, debug=False)
    bf16 = mybir.dt.bfloat16
    if cfg["mm1_dtype"] == "bf16x2":
        qT_d = nc.dram_tensor("qT2", (B_LOC, S, 2, L), bf16, kind="ExternalInput")
        kph_d = nc.dram_tensor("kph", (B_LOC, S, S), bf16, kind="ExternalInput")
        kpl_d = nc.dram_tensor("kpl", (B_LOC, S, S), bf16, kind="ExternalInput")
    else:
        mm1_dt_glob = f32 if cfg["mm1_dtype"] == "f32" else f32r
        qT_d = nc.dram_tensor("qT", (B_LOC, S, L), mm1_dt_glob, kind="ExternalInput")
    kp_d = nc.dram_tensor("kp", (B_LOC, S, S), f32, kind="ExternalInput")
    v_d = nc.dram_tensor("v", (B_LOC, S, S), f32, kind="ExternalInput")
    ind_d = None
    if cfg["bias_mode"] == "pe":
        ind_d = nc.dram_tensor("ind", (NT, LBLK), f32r, kind="ExternalInput")
    sel16_d = None
    if cfg["dataflow"] == "t" or cfg["rowsum_mode"] == "pe_pt":
        sel16_d = nc.dram_tensor("sel16", (S, NBLK * NBLK), f32r,
                                 kind="ExternalInput")
    out_dt = f32 if cfg["out_dtype"] == "f32" else mybir.dt.bfloat16
    outT_d = nc.dram_tensor("outT", (B_LOC, S, L), out_dt, kind="ExternalOutput")
    if cfg["rowsum_mode"] == "dve" and cfg["dataflow"] == "lsoft":
        rs_d = nc.dram_tensor("rs", (B_LOC, S, L // S), f32, kind="ExternalOutput")
    elif cfg["dataflow"] == "t" or cfg["rowsum_mode"] == "pe_pt":
        rs_d = nc.dram_tensor("rs", (B_LOC, NBLK, LBLK), f32, kind="ExternalOutput")
    else:
        rs_d = nc.dram_tensor("rs", (B_LOC, L), f32, kind="ExternalOutput")

    from concourse import bass_isa
    Exp = mybir.ActivationFunctionType.Exp
    AX = mybir.AxisListType.X
    MAX = mybir.AluOpType.max
    ADD = mybir.AluOpType.add

    with tile.TileContext(nc) as tc:
        with (
            tc.tile_pool(name="const", bufs=1) as constp,
            tc.tile_pool(name="qt", bufs=cfg["qt_bufs"]) as qtp,
            tc.tile_pool(name="p", bufs=cfg["p_bufs"]) as pp,
            tc.tile_pool(name="pt", bufs=cfg["pt_bufs"]) as ptp,
            tc.tile_pool(name="nm", bufs=cfg["nm_bufs"]) as nmp,
            tc.tile_pool(name="rss", bufs=cfg["rs_bufs"]) as rsp,
            tc.tile_pool(name="oc", bufs=cfg["oc_bufs"]) as ocp,
            tc.tile_pool(name="scps", bufs=cfg["sc_bufs"], space="PSUM") as scps,
            tc.tile_pool(name="ptps", bufs=cfg["ptps_bufs"], space="PSUM") as ptps,
            tc.tile_pool(name="ops", bufs=cfg["o_bufs"], space="PSUM") as ops,
            tc.tile_pool(name="rsps", bufs=cfg["rsps_bufs"], space="PSUM") as rspsp,
            tc.tile_pool(name="auxps", bufs=1, space="PSUM") as auxps,
        ):
            ident = constp.tile([128, 128], f32, tag="ident")
            make_identity(nc, ident[:])
            kp_sb = constp.tile([128, B_LOC * 128], f32, tag="kp")
            v_sb = constp.tile([128, B_LOC * 128], f32, tag="v")
            v_r = constp.tile([128, B_LOC * 128], f32r, tag="vr")
            ind_r = None
            if cfg["bias_mode"] == "pe":
                ind_r = constp.tile([NT, LBLK], f32r, tag="ind")
                nc.sync.dma_start(ind_r[:], ind_d[:])
            ones_r = None
            if cfg["rowsum_mode"] == "pe" or cfg["dataflow"] == "t":
                ones_f = constp.tile([128, 1], f32, tag="ones_f")
                ones_r = constp.tile([128, 1], f32r, tag="ones")
                nc.gpsimd.memset(ones_f[:], 1.0)
                nc.vector.tensor_copy(ones_r[:], ones_f[:])
            neg_inv_r = None
            sel16_r = None
            if cfg["rowsum_mode"] == "pe_pt" and cfg["dataflow"] != "t":
                sel16_r = constp.tile([128, NBLK * NBLK], f32r, tag="sel16")
                nc.sync.dma_start(sel16_r[:], sel16_d[:])
            if cfg["dataflow"] == "t":
                neg_inv_f = constp.tile([128, 128], f32, tag="ninv_f")
                neg_inv_r = constp.tile([128, 128], f32r, tag="ninv")
                nc.gpsimd.memset(neg_inv_f[:], -1.0 / 128.0)
                nc.vector.tensor_copy(neg_inv_r[:], neg_inv_f[:])
                sel16_r = constp.tile([128, NBLK * NBLK], f32r, tag="sel16")
                nc.sync.dma_start(sel16_r[:], sel16_d[:])
            for b in range(B_LOC):
                nc.sync.dma_start(kp_sb[:, ts(b, 128)], kp_d[b])
                nc.sync.dma_start(v_sb[:, ts(b, 128)], v_d[b])
            nc.vector.tensor_copy(v_r[:], v_sb[:])
            kp_r = None
            if cfg["mm1_dtype"] == "f32r":
                kp_r = constp.tile([128, B_LOC * 128], f32r, tag="kpr")
                nc.vector.tensor_copy(kp_r[:], kp_sb[:])
            kph_sb = kpl_sb = None
            if cfg["mm1_dtype"] == "bf16x2":
                bf16_ = mybir.dt.bfloat16
                kph_sb = constp.tile([128, B_LOC * 128], bf16_, tag="kph")
                kpl_sb = constp.tile([128, B_LOC * 128], bf16_, tag="kpl")
                for b in range(B_LOC):
                    nc.sync.dma_start(kph_sb[:, ts(b, 128)], kph_d[b])
                    nc.sync.dma_start(kpl_sb[:, ts(b, 128)], kpl_d[b])

            def t_block(b, blk, rs_stage, rs_ps_holder):
                l0 = blk * LBLK
                sc = scps.tile([128, LBLK], f32, tag="sc")
                if cfg["mm1_dtype"] == "bf16x2":
                    bf16_ = mybir.dt.bfloat16
                    qt2 = qtp.tile([128, 2 * LBLK], bf16_, tag="qt")
                    nc.sync.dma_start(
                        qt2[:].rearrange("p (h l) -> p h l", h=2),
                        qT_d[b, :, :, l0:l0 + LBLK])
                    qh = qt2[:, 0:LBLK]
                    ql = qt2[:, LBLK:2 * LBLK]
                    nc.tensor.matmul(sc[:], kph_sb[:, ts(b, 128)], qh,
                                     start=True, stop=False)
                    nc.tensor.matmul(sc[:], kpl_sb[:, ts(b, 128)], qh,
                                     start=False, stop=False)
                    nc.tensor.matmul(sc[:], kph_sb[:, ts(b, 128)], ql,
                                     start=False, stop=False)
                else:
                    mm1_dt = f32 if cfg["mm1_dtype"] == "f32" else f32r
                    kp_use = kp_sb if cfg["mm1_dtype"] == "f32" else kp_r
                    qt = qtp.tile([128, LBLK], mm1_dt, tag="qt")
                    nc.sync.dma_start(qt[:], qT_d[b, :, l0:l0 + LBLK])
                    nc.tensor.matmul(sc[:], kp_use[:, ts(b, 128)], qt[:],
                                     start=True, stop=False)
                scs = pp.tile([128, LBLK], f32, tag="scs")
                if cfg["scs_copy_engine"] == "dve":
                    nc.vector.tensor_copy(scs[:], sc[:])
                else:
                    nc.scalar.copy(scs[:], sc[:])
                mx_dt = f32r if cfg["max_out_dtype"] == "f32r" else f32
                mxr = ptp.tile([128, LBLK], mx_dt, tag="mxr")
                nc.gpsimd.partition_all_reduce(
                    mxr[:], scs[:], 128, bass_isa.ReduceOp.max)
                nc.tensor.matmul(sc[:], neg_inv_r[:], mxr[:],
                                 start=False, stop=True)
                pt = ptp.tile([128, LBLK], f32r, tag="pt")
                nc.scalar.activation(pt[:], sc[:], Exp, bias=0.0, scale=1.0)
                if blk == 0:
                    rs_ps_new = rspsp.tile([NBLK, LBLK], f32, tag="rsps")
                    rs_ps_holder[0] = rs_ps_new
                rs_ps = rs_ps_holder[0]
                nc.tensor.matmul(rs_ps[:], sel16_r[:, blk * NBLK:(blk + 1) * NBLK],
                                 pt[:], start=(blk == 0), stop=(blk == NBLK - 1))
                if blk == NBLK - 1:
                    rsx = rsp.tile([NBLK, LBLK], f32, tag="rsx")
                    nc.vector.tensor_copy(rsx[:], rs_ps[:])
                    nc.sync.dma_start(rs_d[b], rsx[:])
                op_t = ops.tile([128, LBLK], f32, tag="op")
                nc.tensor.matmul(op_t[:], v_r[:, ts(b, 128)], pt[:],
                                 start=True, stop=True)
                oc = ocp.tile([128, LBLK], out_dt, tag="oc")
                oce = cfg["out_copy_engine"]
                if oce in ("alt", "act") or oce.startswith("mix"):
                    nc.scalar.copy(oc[:], op_t[:])
                else:
                    nc.vector.tensor_copy(oc[:], op_t[:])
                st = {"pool": nc.gpsimd, "act": nc.scalar,
                      "dve": nc.vector, "sp": nc.sync}[cfg["store_engine"]]
                st.dma_start(outT_d[b, :, l0:l0 + LBLK], oc[:])

            def t_body(_iv=None):
                for b in range(B_LOC):
                    holder = [None]
                    for blk in range(NBLK):
                        t_block(b, blk, None, holder)

            def body(_iv=None):
                if cfg["dataflow"] == "t":
                    return t_body(_iv)
                for b in range(B_LOC):
                    mode = cfg["rowsum_mode"]
                    rs16_holder = [None]
                    rs_stage = None
                    if mode == "dve":
                        rs_stage = rsp.tile([128, L // S], f32, tag="rss")
                    elif mode == "pool":
                        rs_stage = rsp.tile([128, L], f32, tag="rss")
                    for blk in range(NBLK):
                        l0 = blk * LBLK
                        mm1_dt = f32 if cfg["mm1_dtype"] == "f32" else f32r
                        qt = qtp.tile([128, LBLK], mm1_dt, tag="qt")
                        nc.sync.dma_start(qt[:], qT_d[b, :, l0:l0 + LBLK])
                        sc = scps.tile([128, LBLK], f32, tag="sc")
                        for ti in range(NT):
                            nc.tensor.matmul(
                                sc[:, ts(ti, 128)], qt[:, ts(ti, 128)],
                                (kp_sb if cfg["mm1_dtype"] == "f32" else kp_r)[:, ts(b, 128)],
                                start=True,
                                stop=(cfg["bias_mode"] == "act"),
                                skip_group_check=(cfg["bias_mode"] == "pe"))
                        nm = nmp.tile([128, NT], f32, tag="nm")
                        nc.vector.tensor_reduce(
                            nm[:], sc[:].rearrange("p (t s) -> p t s", t=NT),
                            axis=AX, op=MAX, negate=True)
                        p = pp.tile([128, LBLK], f32, tag="p")
                        if cfg["bias_mode"] == "act":
                            for ti in range(NT):
                                nc.scalar.activation(
                                    p[:, ts(ti, 128)], sc[:, ts(ti, 128)], Exp,
                                    bias=nm[:, ti:ti + 1], scale=1.0)
                        else:
                            nmt_ps = auxps.tile([NT, 128], f32, tag="nmt")
                            nc.tensor.transpose(nmt_ps[:], nm[:], ident[:])
                            nmt = nmp.tile([NT, 128], f32r, tag="nmtr")
                            if cfg["nm_copy_engine"] == "dve":
                                nc.vector.tensor_copy(nmt[:], nmt_ps[:])
                            else:
                                nc.scalar.copy(nmt[:], nmt_ps[:])
                            nc.tensor.matmul(sc[:], nmt[:], ind_r[:],
                                             start=False, stop=True,
                                             skip_group_check=True)
                            nc.scalar.activation(p[:], sc[:], Exp,
                                                 bias=0.0, scale=1.0)
                        if cfg["rowsum_mode"] == "dve":
                            nc.vector.tensor_reduce(
                                rs_stage[:, blk * NT:(blk + 1) * NT],
                                p[:].rearrange("p (t s) -> p t s", t=NT),
                                axis=AX, op=ADD)
                        ptps_t = ptps.tile([128, LBLK], f32, tag="ptps")
                        for ti in range(NT):
                            nc.tensor.transpose(
                                ptps_t[:, ts(ti, 128)], p[:, ts(ti, 128)],
                                ident[:])
                        pt = ptp.tile([128, LBLK], f32r, tag="pt")
                        pce = cfg["pt_copy_engine"]
                        if pce == "alt":
                            pce = "dve" if blk % 2 == 0 else "act"
                        elif pce.startswith("mix"):
                            n, m = pce[3:].split("of")
                            pce = "dve" if blk % int(m) < int(n) else "act"
                        if pce == "dve":
                            nc.vector.tensor_copy(pt[:], ptps_t[:])
                        else:
                            nc.scalar.copy(pt[:], ptps_t[:])
                        if cfg["rowsum_mode"] == "pool":
                            nc.gpsimd.partition_all_reduce(
                                rs_stage[:, blk * LBLK:(blk + 1) * LBLK],
                                pt[:], 128, bass_isa.ReduceOp.add)
                        elif cfg["rowsum_mode"] == "pe":
                            if blk % 4 == 0:
                                rs_ps = rspsp.tile([128, LBLK], f32, tag="rsps")
                            j = blk % 4
                            nc.tensor.matmul(
                                rs_ps[32 * j:32 * j + 1, :], ones_r[:], pt[:],
                                start=True, stop=True,
                                tile_position=(0, 32 * j))
                            if j == 3:
                                nc.vector.tensor_copy(
                                    rs_stage[(blk - 3) // 4 * 4:(blk - 3) // 4 * 4 + 4, :].rearrange("a b -> a b"),
                                    rs_ps[:].rearrange("(a c) b -> a c b", c=32)[:, 0:1, :].rearrange("a c b -> (a c) b"))
                        if cfg["rowsum_mode"] == "pe_pt":
                            if blk == 0:
                                rs16_new = rspsp.tile([NBLK, LBLK], f32,
                                                      tag="rsps")
                                rs16_holder[0] = rs16_new
                            rs16 = rs16_holder[0]
                            nc.tensor.matmul(
                                rs16[:],
                                sel16_r[:, blk * NBLK:(blk + 1) * NBLK],
                                pt[:], start=(blk == 0),
                                stop=(blk == NBLK - 1))
                            if blk == NBLK - 1:
                                rsx = rsp.tile([NBLK, LBLK], f32, tag="rsx")
                                nc.vector.tensor_copy(rsx[:], rs16[:])
                                nc.sync.dma_start(rs_d[b], rsx[:])
                        op_t = ops.tile([128, LBLK], f32, tag="op")
                        nc.tensor.matmul(
                            op_t[:], v_r[:, ts(b, 128)], pt[:],
                            start=True, stop=True)
                        oc = ocp.tile([128, LBLK], out_dt, tag="oc")
                        oce = cfg["out_copy_engine"]
                        if oce == "alt":
                            oce = "act" if blk % 2 == 0 else "dve"
                        elif oce.startswith("mix"):
                            n, m = oce[3:].split("of")
                            oce = "dve" if blk % int(m) < int(n) else "act"
                        if oce == "act":
                            nc.scalar.copy(oc[:], op_t[:])
                        else:
                            nc.vector.tensor_copy(oc[:], op_t[:])
                        st = {"pool": nc.gpsimd, "act": nc.scalar,
                              "dve": nc.vector, "sp": nc.sync}[cfg["store_engine"]]
                        st.dma_start(outT_d[b, :, l0:l0 + LBLK], oc[:])
                    if cfg["rowsum_mode"] == "dve":
                        nc.gpsimd.dma_start(rs_d[b], rs_stage[:])
                    elif cfg["rowsum_mode"] == "pool":
                        nc.sync.dma_start(rs_d[b], rs_stage[0:1, :].rearrange("a b -> (a b)"))

            if repeat == 1:
                body()
            else:
                un = cfg["t2_unroll"]
                while repeat % un:
                    un -= 1
                with tc.For_i(0, repeat // un, 1,
                              staggered_reset=cfg["t2_sreset"]) as _i:
                    for _ in range(un):
                        body(_i)

    nc.compile()
    return nc


def _make_runner(repeat=1, cfg=None):
    """Compile (once) and return fn(in_maps) -> list[dict] per core."""
    key = (repeat, tuple(sorted((cfg or {}).items())))
    if key in _RUNNER_CACHE:
        return _RUNNER_CACHE[key]

    import jax
    import concourse.mybir as mybir
    from concourse import bass2jax
    from concourse.bass2jax import _bass_exec_p, partition_id_tensor
    from jax.sharding import Mesh, NamedSharding, PartitionSpec
    from jax.experimental.shard_map import shard_map

    nc = _build_nc(repeat, cfg)
    bass2jax.install_neuronx_cc_hook()

    in_names, out_names, out_avals, zero_shapes = [], [], [], []
    for alloc in nc.m.functions[0].allocations:
        if not isinstance(alloc, mybir.MemoryLocationSet):
            continue
        name = alloc.memorylocations[0].name
        if alloc.kind == "ExternalInput":
            if nc.partition_id_tensor is None or name != nc.partition_id_tensor.name:
                in_names.append(name)
        elif alloc.kind == "ExternalOutput":
            out_names.append(name)
            shape = tuple(alloc.tensor_shape)
            dtype = mybir.dt.np(alloc.dtype)
            out_avals.append(jax.core.ShapedArray(shape, dtype))
            zero_shapes.append((shape, dtype))
    n_params = len(in_names)
    pid_name = nc.partition_id_tensor.name if nc.partition_id_tensor else None
    names_for_bind = in_names + out_names + ([pid_name] if pid_name else [])

    def _body(*args):
        operands = list(args)
        if pid_name:
            operands.append(partition_id_tensor())
        outs = _bass_exec_p.bind(
            *operands,
            out_avals=tuple(out_avals),
            in_names=tuple(names_for_bind),
            out_names=tuple(out_names),
            lowering_input_output_aliases=(),
            sim_require_finite=True,
            sim_require_nnan=True,
            nc=nc,
        )
        return tuple(outs)

    devices = jax.devices()[:N_CORES]
    mesh = Mesh(np.asarray(devices), ("core",))
    nspec = n_params + len(out_names)
    fn = jax.jit(
        shard_map(_body, mesh=mesh,
                  in_specs=(PartitionSpec("core"),) * nspec,
                  out_specs=(PartitionSpec("core"),) * len(out_names),
                  check_rep=False),
        keep_unused=True)
    sharding = NamedSharding(mesh, PartitionSpec("core"))

    def run(in_maps):
        import jax as _jax
        concat_in = [
            np.concatenate([np.asarray(m[name]) for m in in_maps], axis=0)
            for name in in_names
        ]
        zeros = [np.zeros((N_CORES * s[0],) + tuple(s[1:]), d)
                 for (s, d) in zero_shapes]
        dev_in = [_jax.device_put(a, sharding) for a in concat_in + zeros]
        out_arrs = fn(*dev_in)
        _jax.block_until_ready(out_arrs)
        return [
            {name: np.asarray(out_arrs[i]).reshape(
                (N_CORES,) + tuple(out_avals[i].shape))[c]
             for i, name in enumerate(out_names)}
            for c in range(N_CORES)
        ], (fn, dev_in)

    _RUNNER_CACHE[key] = run
    return run


_VCOL = {}


def _prep_inputs_t2(queries, keys, values, cfg=None):
    cfg = {**CFG, **(cfg or {})}
    import ml_dtypes
    bf = ml_dtypes.bfloat16
    qT = queries.transpose(0, 2, 1)                            # (B, E, L)
    qT16 = np.ascontiguousarray(qT.astype(np.float16))
    k16 = np.ascontiguousarray(keys.astype(np.float16))        # lhsT = keys
    v = np.ascontiguousarray(values.astype(np.float32))
    if cfg["t2_vones"]:
        # ones-column trick: y = V^-1 1; col c* = argmax |y_c|;
        # v[:,c*] := 1 on device, host recovers out_c* = alpha.out + beta*rs
        v64 = values.astype(np.float64)
        cols = np.empty(B, np.int64)
        ys = np.empty((B, S), np.float64)
        for b in range(B):
            y = np.linalg.solve(v64[b], np.ones(S))
            c = int(np.argmax(np.abs(y)))
            cols[b] = c
            ys[b] = y
            v[b, :, c] = 1.0
        _VCOL["cols"] = cols
        _VCOL["ys"] = ys
    qsum = queries.sum(axis=2, dtype=np.float64).astype(np.float32)  # (B, L)
    ksum = keys.sum(axis=2, dtype=np.float64).astype(np.float32)     # (B, S)
    qh = qsum.astype(bf).astype(np.float32)
    ql = qsum - qh
    kh = ksum.astype(bf).astype(np.float32)
    kl = ksum - kh
    Kmax = ksum.max(axis=1, keepdims=True)
    Kmin = ksum.min(axis=1, keepdims=True)
    M = np.maximum(qsum * Kmax, qsum * Kmin) + 10.0
    qrows = np.ascontiguousarray(
        np.stack([qh, ql, qh, -M, ql], axis=1).astype(np.float16))   # (B,5,L)
    ones = np.ones_like(kh)
    kr5 = np.ascontiguousarray(
        np.stack([kh, kh, kl, ones, kl], axis=1).astype(np.float16))  # (B,5,S)
    sel16 = np.zeros((S, NBLK * NBLK), np.float32)
    for j in range(NBLK):
        sel16[:, j * NBLK + j] = 1.0
    in_maps = []
    for c in range(N_CORES):
        sl = slice(c * B_LOC, (c + 1) * B_LOC)
        m = {"qT16": qT16[sl], "k16": k16[sl], "vr": v[sl]}
        if cfg["t2_b4"]:
            qr5 = qrows[sl].transpose(1, 0, 2).reshape(5, B_LOC * L)
            qr101 = np.zeros((128, B_LOC * L), qrows.dtype)
            kr5f = kr5[sl].transpose(1, 0, 2).reshape(5, B_LOC * S)
            kr101 = np.zeros((128, B_LOC * S), kr5.dtype)
            for t in range(4):
                qr101[32 * t:32 * t + 5] = qr5
                kr101[32 * t:32 * t + 5] = kr5f
            m["qrows101"] = np.ascontiguousarray(qr101)
            m["kr5x"] = np.ascontiguousarray(kr101)
        elif cfg["t2_qr_const"]:
            m["qrows5"] = np.ascontiguousarray(
                qrows[sl].transpose(1, 0, 2).reshape(5, B_LOC * L))
            m["kr5"] = kr5[sl]
        else:
            m["qrows"] = qrows[sl]
            m["kr5"] = kr5[sl]
        if not cfg["t2_vones"]:
            m["sel16"] = sel16
        in_maps.append(m)
    return in_maps


def _prep_inputs(queries, keys, values, cfg=None):
    cfg = {**CFG, **(cfg or {})}
    if cfg["dataflow"] == "t2":
        return _prep_inputs_t2(queries, keys, values, cfg)
    qT = np.ascontiguousarray(queries.transpose(0, 2, 1))      # (B, E, L)
    kp = keys + keys.sum(axis=2)[:, None, :]                   # k' = k + 1*ksum
    kp = np.ascontiguousarray(kp.astype(np.float32))
    v = np.ascontiguousarray(values.astype(np.float32))
    qT2 = kph = kpl = None
    if cfg["mm1_dtype"] == "bf16x2":
        import ml_dtypes
        bf = ml_dtypes.bfloat16
        qTh = qT.astype(bf)
        qTl = (qT - qTh.astype(np.float32)).astype(bf)
        qT2 = np.ascontiguousarray(np.stack([qTh, qTl], axis=2))
        kph = kp.astype(bf)
        kpl = np.ascontiguousarray((kp - kph.astype(np.float32)).astype(bf))
        kph = np.ascontiguousarray(kph)
    ind = np.zeros((NT, LBLK), np.float32)
    for ti in range(NT):
        ind[ti, ti * 128:(ti + 1) * 128] = 1.0
    sel16 = np.zeros((S, NBLK * NBLK), np.float32)
    for j in range(NBLK):
        sel16[:, j * NBLK + j] = 1.0
    in_maps = []
    for c in range(N_CORES):
        sl = slice(c * B_LOC, (c + 1) * B_LOC)
        m = {"qT": qT[sl], "kp": kp[sl], "v": v[sl], "ind": ind,
             "sel16": sel16}
        if qT2 is not None:
            m.update({"qT2": qT2[sl], "kph": kph[sl], "kpl": kpl[sl]})
        in_maps.append(m)
    return in_maps


def _assemble(results, cfg=None):
    cfg = {**CFG, **(cfg or {})}
    if cfg["dataflow"] == "t2" and cfg["t2_vones"]:
        return _assemble_t2_vones(results)
    out = np.empty((B, L, S), dtype=np.float32)
    for c in range(N_CORES):
        outT = results[c]["outT"]          # (B_LOC, S, L)  = out^T per batch
        rs = results[c]["rs"]              # (B_LOC, 128, L//128) rowsums
        for b in range(B_LOC):
            if cfg["rowsum_mode"] == "dve" and cfg["dataflow"] == "lsoft":
                rsum = rs[b].T.reshape(L)  # rowsum[l]
            else:
                rsum = rs[b].reshape(L)
            if cfg["dataflow"] == "t2":
                num = outT[b].astype(np.float64).T
                out[c * B_LOC + b] = (num / rsum.astype(np.float64)[:, None]
                                      ).astype(np.float32)
            else:
                out[c * B_LOC + b] = outT[b].T / rsum[:, None]
    return out.reshape(B, 1, L, S)


def _assemble_t2_vones(results):
    out = np.empty((B, L, S), dtype=np.float32)
    cols = _VCOL["cols"]
    ys = _VCOL["ys"]
    for c in range(N_CORES):
        outT = results[c]["outT"]              # (B_LOC, S, L) bf16
        for bl in range(B_LOC):
            b = c * B_LOC + bl
            oT = outT[bl].astype(np.float64)   # (S, L) rows = d
            cc = cols[b]
            y = ys[b]
            rs = oT[cc]                        # rowsum row
            beta = 1.0 / y[cc]
            alpha = -y * beta
            alpha[cc] = 0.0
            rec = alpha @ oT + beta * rs       # recovered out_cc (L,)
            o = oT / rs[None, :]
            o[cc] = rec / rs
            out[b] = o.T.astype(np.float32)
    return out.reshape(B, 1, L, S)


def kernel(queries, keys, values):
    run = _make_runner(repeat=1)
    in_maps = _prep_inputs(queries, keys, values)
    results, _ = run(in_maps)
    return _assemble(results)

